# revision 17
# baseline (speedup 1.0000x reference)
"""Trainium2 Bass kernel for per-position head-attention (nn_DariushFlashAttention2).

Math (per batch b, sequence position s):
    Q = q[b,s].reshape(H=32, D=128); K, V likewise
    logits = Q @ K.T / sqrt(D)          # [32, 32] attention over HEADS
    W = softmax(logits, axis=-1)
    out[b,s] = (W @ V).reshape(H*D)

Every one of the B*S = 8192 positions is independent, so we shard positions
across the 8 NeuronCores (1024 positions each) and run one SPMD program.

Device strategy (per core):
  - Positions are packed 4-per-"group" onto the 128 SBUF partitions
    (partition = 4*32 = pos_in_group x head).
  - Host pre-transposes q,k into [d, (pos,h)] layout and pre-casts to
    fp8-e3m4 (4 mantissa bits keep the overall rel-err ~1.7e-2, under the
    2e-2 gate), packed side by side in ONE dram tensor so each chunk loads
    with a single 8KB-row DMA.  v stays fp16 with a ones-column per group
    (emits the softmax denominator from the same WV matmul).
  - QK: per position one col-tiled fp8 matmul (tile_position=(0,32j),
    K=128(d), M=32(k-heads), N=32(q-heads)) -> psum block-row for that
    position only; 4 quads of groups share one [128,512] psum bank.
  - One exp() per 16 groups on ScalarE over the whole [128,512] psum tile.
  - WV: per position a (32j,32j) sub-array matmul; V is stored
    [(pos,g), d|1] so the same matmul emits the denominator in its last
    column.  4 groups of WV output live in one [128,1024] 2-bank psum tile.
  - One batched reciprocal per 4 groups; normalize-and-evacuate PSUM with
    one broadcast tensor_tensor per 4 groups on VectorE (3 of 4 tiles) or
    per-group scaled copies on ScalarE (1 of 4 tiles) to balance engines.
  - Output halves drain via the GpSimd HWDGE ring so out-DMAs never
    contend with input prefetch or ScalarE dispatch.
"""

import numpy as np

B, S, H, D = 2, 4096, 32, 128
NCORES = 8
POS = B * S                  # 8192 positions total
PPC = POS // NCORES          # 1024 positions per core
GP = 4                       # positions per group (4*32 heads = 128 partitions)
NG = 32                      # groups per chunk
CHUNK_POS = GP * NG          # 128 positions per chunk
NCHUNK = PPC // CHUNK_POS    # 8 chunks per core
VCOL = D + 1                 # v columns per group incl. ones column
C = NG * D                   # 4096 q (or k) columns per chunk

_SCALE = float(1.0 / np.sqrt(D))

_program = None  # cached compiled Bass program


def _build_program():
    import concourse.bacc as bacc
    import concourse.mybir as mybir
    from concourse.tile import TileContext

    fp32 = mybir.dt.float32
    fp16 = mybir.dt.float16
    fp8 = mybir.dt.float8e3

    nc = bacc.Bacc()
    qk = nc.dram_tensor("qk", [NCHUNK, 128, 2 * C], fp8, kind="ExternalInput")
    vp = nc.dram_tensor("vp", [NCHUNK, 128, NG * VCOL], fp16, kind="ExternalInput")
    out = nc.dram_tensor("out", [NCHUNK, 128, C], fp16, kind="ExternalOutput")

    NHALF = NCHUNK * 2
    with TileContext(nc) as tc:
        with (
            tc.tile_pool(name="qk_in", bufs=3) as qk_pool,
            tc.tile_pool(name="v_in", bufs=3) as v_pool,
            tc.tile_pool(name="o_out", bufs=3) as o_pool,
            tc.tile_pool(name="exp", bufs=3) as exp_pool,
            tc.tile_pool(name="small", bufs=8) as small_pool,
            tc.tile_pool(name="psl", bufs=2, space="PSUM") as psl_pool,
            tc.tile_pool(name="pso", bufs=3, space="PSUM") as pso_pool,
        ):
            chunk_tiles = {}   # n -> (qk_t, vp_t, out_t)
            half_state = {}    # hh -> exp_sb

            def emit_load(n):
                qk_t = qk_pool.tile([128, 2 * C], fp8, tag="qk")
                vp_t = v_pool.tile([128, NG * VCOL], fp16, tag="vp")
                nc.sync.dma_start(out=qk_t, in_=qk[n, :, :])
                nc.sync.dma_start(out=vp_t, in_=vp[n, :, :])
                out_t = o_pool.tile([128, C], fp16, tag="out")
                chunk_tiles[n] = (qk_t, vp_t, out_t)

            def emit_qk_quad(hh, qq):
                n, half = divmod(hh, 2)
                qk_t, _, _ = chunk_tiles[n]
                if qq == 0:
                    half_state[hh] = psl_pool.tile(
                        [128, 512], fp32, tag="psl", name=f"psl_{hh}")
                psl = half_state[hh]
                for t in range(4):           # group within quad
                    g = half * 16 + qq * 4 + t
                    for j in range(GP):      # position within group
                        c = slice(g * D + 32 * j, g * D + 32 * j + 32)
                        ck = slice(C + g * D + 32 * j, C + g * D + 32 * j + 32)
                        nc.tensor.matmul(
                            psl[32 * j:32 * j + 32,
                                128 * qq + 32 * t:128 * qq + 32 * t + 32],
                            qk_t[:, ck],
                            qk_t[:, c],
                            start=True, stop=True,
                            tile_position=(0, 32 * j),
                        )

            def emit_exp(hh):
                psl = half_state[hh]
                exp_sb = exp_pool.tile([128, 512], fp16, tag="exp_sb",
                                       name=f"exp_{hh}")
                nc.scalar.activation(
                    exp_sb, psl, mybir.ActivationFunctionType.Exp,
                    scale=_SCALE,
                )
                half_state[hh] = exp_sb

            def emit_wv_quad(hh, qq):
                n, half = divmod(hh, 2)
                _, vp_t, out_t = chunk_tiles[n]
                exp_sb = half_state[hh]
                psum_o = pso_pool.tile([128, 1024], fp32, tag="pso",
                                       name=f"pso_{hh}_{qq}")
                for p2 in range(2):          # pair of groups (one psum bank)
                    for u in range(2):
                        g = half * 16 + qq * 4 + p2 * 2 + u
                        t = p2 * 2 + u
                        for j in range(GP):
                            r = slice(32 * j, 32 * j + 32)
                            nc.tensor.matmul(
                                psum_o[r, 512 * p2 + 129 * u:
                                       512 * p2 + 129 * u + 129],
                                exp_sb[r, 128 * qq + 32 * t:
                                       128 * qq + 32 * t + 32],
                                vp_t[r, g * VCOL:(g + 1) * VCOL],
                                start=True, stop=True,
                                tile_position=(32 * j, 32 * j),
                            )
                recip = small_pool.tile([128, 4], fp32, tag="recip",
                                        name=f"recip_{hh}_{qq}")
                pv = psum_o.rearrange("p (a r) -> p a r", a=2)[:, :, :258]
                pv = pv.rearrange("p a (u c) -> p a u c", u=2)
                rv = recip.rearrange("p (a u) -> p a u", a=2)
                nc.vector.reciprocal(rv, pv[:, :, :, D])

                g0 = half * 16 + qq * 4
                if False:
                    # ScalarE path: per-group scaled copies.  Quad 1, not 0 or
                    # 3: with pso bufs=3, quad 3 reuses quad 0's psum buffer,
                    # so quad 0 must be freed by the fast VectorE path; the
                    # slow scalar chain on quad 1 has until next slot's quad 0.
                    for i in range(4):
                        g = g0 + i
                        src = psum_o[:, 512 * (i // 2) + 129 * (i % 2):
                                     512 * (i // 2) + 129 * (i % 2) + D]
                        nc.scalar.activation(
                            out_t[:, g * D:(g + 1) * D], src,
                            mybir.ActivationFunctionType.Copy,
                            scale=recip[:, i:i + 1],
                        )
                else:
                    # VectorE path: one broadcast multiply for 4 groups
                    dst = out_t[:, g0 * D:(g0 + 4) * D].rearrange(
                        "p (a u c) -> p a u c", a=2, u=2)
                    mul = rv.unsqueeze(3).broadcast_to((128, 2, 2, D))
                    nc.vector.tensor_tensor(
                        dst, pv[:, :, :, 0:D], mul,
                        op=mybir.AluOpType.mult,
                    )

            def emit_drain(hh):
                n, half = divmod(hh, 2)
                _, _, out_t = chunk_tiles[n]
                half_state.pop(hh)
                half_c = C // 2
                nc.gpsimd.dma_start(
                    out=out[n, :, half * half_c:(half + 1) * half_c],
                    in_=out_t[:, half * half_c:(half + 1) * half_c])

            # Software pipeline with quad interleave: the PE alternates
            # QK-quad(hh+1) and WV-quad(hh).  The QK matmuls between two WV
            # quads cover the recip->evacuate->psum-free latency, and the
            # weight-load pipe prefetches ahead during the other stream's
            # matmuls, keeping the 4-way column-tiled fast mode engaged.
            # Output drains are deferred DRAIN_DELAY halves so the input
            # prefetch phase gets the full DMA bandwidth; drains then fill
            # the DMA-idle compute tail.
            DRAIN_DELAY = 1
            for hh in range(NHALF + DRAIN_DELAY + 1):
                n, half = divmod(hh, 2)
                if half == 0 and hh < NHALF:
                    emit_load(n)
                for qq in range(4):
                    if hh < NHALF:
                        emit_qk_quad(hh, qq)
                    if 1 <= hh <= NHALF:
                        emit_wv_quad(hh - 1, qq)
                if hh < NHALF:
                    emit_exp(hh)
                if 0 <= hh - DRAIN_DELAY < NHALF:
                    emit_drain(hh - DRAIN_DELAY)

    nc.compile()
    return nc


def _host_pack(q, k, v):
    """Build per-core device input arrays from full fp32 inputs."""
    import ml_dtypes
    f8 = ml_dtypes.float8_e3m4

    qf = np.ascontiguousarray(q, dtype=np.float32).reshape(POS, H, D)
    kf = np.ascontiguousarray(k, dtype=np.float32).reshape(POS, H, D)
    vf = np.ascontiguousarray(v, dtype=np.float32).reshape(POS, H, D)

    nchunk_tot = POS // CHUNK_POS
    # q,k: [chunk, group, i, h, d] -> [chunk, d, (group, i, h)]
    def to_qt(x):
        x = x.reshape(nchunk_tot, NG, GP, H, D)
        x = x.transpose(0, 4, 1, 2, 3)
        return np.ascontiguousarray(x.reshape(nchunk_tot, D, NG * GP * H)).astype(f8)

    qk_all = np.concatenate([to_qt(qf), to_qt(kf)], axis=2)

    # v: [chunk, group, i, gh, d] -> [chunk, (i,gh), (group, d|1)]
    vv = vf.reshape(nchunk_tot, NG, GP, H, D).transpose(0, 2, 3, 1, 4)
    vp_all = np.ones((nchunk_tot, GP, H, NG, VCOL), dtype=np.float32)
    vp_all[..., :D] = vv
    vp_all = np.ascontiguousarray(
        vp_all.reshape(nchunk_tot, GP * H, NG * VCOL)
    ).astype(np.float16)

    in_maps = []
    for c in range(NCORES):
        sl = slice(c * NCHUNK, (c + 1) * NCHUNK)
        in_maps.append({
            "qk": np.ascontiguousarray(qk_all[sl]),
            "vp": np.ascontiguousarray(vp_all[sl]),
        })
    return in_maps


def _host_unpack(outs):
    """Per-core [NCHUNK, 128, C] fp16 -> full [B, S, H*D] fp32."""
    full = np.concatenate(outs, axis=0).astype(np.float32)
    nchunk_tot = POS // CHUNK_POS
    full = full.reshape(nchunk_tot, GP, H, NG, D)   # [chunk, i, h, g, d]
    full = full.transpose(0, 3, 1, 2, 4)            # [chunk, g, i, h, d]
    return np.ascontiguousarray(full.reshape(B, S, H * D))


def kernel(q, k, v, _trace=False):
    global _program
    from concourse.bass_utils import run_bass_kernel_spmd

    if _program is None:
        _program = _build_program()

    in_maps = _host_pack(q, k, v)
    res = run_bass_kernel_spmd(_program, in_maps, list(range(NCORES)), trace=_trace)
    outs = [res.results[c]["out"] for c in range(NCORES)]
    result = _host_unpack(outs)
    if _trace:
        return result, res
    return result


# revision 18
# speedup vs baseline: 1.0897x; 1.0897x over previous
"""Trainium2 Bass kernel for per-position head-attention (nn_DariushFlashAttention2).

Math (per batch b, sequence position s):
    Q = q[b,s].reshape(H=32, D=128); K, V likewise
    logits = Q @ K.T / sqrt(D)          # [32, 32] attention over HEADS
    W = softmax(logits, axis=-1)
    out[b,s] = (W @ V).reshape(H*D)

Every one of the B*S = 8192 positions is independent, so we shard positions
across the 8 NeuronCores (1024 positions each) and run one SPMD program.

Device strategy (per core):
  - Positions are packed 4-per-"group" onto the 128 SBUF partitions
    (partition = 4*32 = pos_in_group x head).
  - Host pre-transposes q,k into [d, (pos,h)] layout and pre-casts to
    fp8-e3m4 (4 mantissa bits keep the overall rel-err ~1.7e-2, under the
    2e-2 gate), packed side by side in ONE dram tensor so each chunk loads
    with a single 8KB-row DMA.  v stays fp16 with a ones-column per group
    (emits the softmax denominator from the same WV matmul).
  - QK: per position one col-tiled fp8 matmul (tile_position=(0,32j),
    K=128(d), M=32(k-heads), N=32(q-heads)) -> psum block-row for that
    position only; 4 quads of groups share one [128,512] psum bank.
  - One exp() per 16 groups on ScalarE over the whole [128,512] psum tile.
  - WV: per position a (32j,32j) sub-array matmul; V is stored
    [(pos,g), d|1] so the same matmul emits the denominator in its last
    column.  4 groups of WV output live in one [128,1024] 2-bank psum tile.
  - One batched reciprocal per 4 groups; normalize-and-evacuate PSUM with
    one broadcast tensor_tensor per 4 groups on VectorE (3 of 4 tiles) or
    per-group scaled copies on ScalarE (1 of 4 tiles) to balance engines.
  - Output halves drain via the GpSimd HWDGE ring so out-DMAs never
    contend with input prefetch or ScalarE dispatch.
"""

import numpy as np

B, S, H, D = 2, 4096, 32, 128
NCORES = 8
POS = B * S                  # 8192 positions total
PPC = POS // NCORES          # 1024 positions per core
GP = 4                       # positions per group (4*32 heads = 128 partitions)
NG = 32                      # groups per chunk
CHUNK_POS = GP * NG          # 128 positions per chunk
NCHUNK = PPC // CHUNK_POS    # 8 chunks per core
VCOL = D + 1                 # v columns per group incl. ones column
C = NG * D                   # 4096 q (or k) columns per chunk

_SCALE = float(1.0 / np.sqrt(D))

_program = None  # cached compiled Bass program


def _build_program():
    import concourse.bacc as bacc
    import concourse.mybir as mybir
    from concourse.tile import TileContext

    fp32 = mybir.dt.float32
    fp16 = mybir.dt.float16
    fp8 = mybir.dt.float8e3

    nc = bacc.Bacc()
    qk = nc.dram_tensor("qk", [NCHUNK, 128, 2 * C], fp8, kind="ExternalInput")
    vp = nc.dram_tensor("vp", [NCHUNK, 128, NG * VCOL], fp16, kind="ExternalInput")
    out = nc.dram_tensor("out", [NCHUNK, 128, C], fp16, kind="ExternalOutput")

    NHALF = NCHUNK * 2
    with TileContext(nc) as tc:
        with (
            tc.tile_pool(name="qk_in", bufs=3) as qk_pool,
            tc.tile_pool(name="v_in", bufs=3) as v_pool,
            tc.tile_pool(name="o_out", bufs=3) as o_pool,
            tc.tile_pool(name="exp", bufs=3) as exp_pool,
            tc.tile_pool(name="small", bufs=8) as small_pool,
            tc.tile_pool(name="psl", bufs=2, space="PSUM") as psl_pool,
            tc.tile_pool(name="pso", bufs=3, space="PSUM") as pso_pool,
        ):
            chunk_tiles = {}   # n -> (qk_t, vp_t, out_t)
            half_state = {}    # hh -> exp_sb

            def emit_load(n):
                qk_t = qk_pool.tile([128, 2 * C], fp8, tag="qk")
                vp_t = v_pool.tile([128, NG * VCOL], fp16, tag="vp")
                nc.sync.dma_start(out=qk_t, in_=qk[n, :, :])
                nc.sync.dma_start(out=vp_t, in_=vp[n, :, :])
                out_t = o_pool.tile([128, C], fp16, tag="out")
                chunk_tiles[n] = (qk_t, vp_t, out_t)

            def emit_qk_quad(hh, qq):
                n, half = divmod(hh, 2)
                qk_t, _, _ = chunk_tiles[n]
                if qq == 0:
                    half_state[hh] = psl_pool.tile(
                        [128, 512], fp32, tag="psl", name=f"psl_{hh}")
                psl = half_state[hh]
                for t in range(4):           # group within quad
                    g = half * 16 + qq * 4 + t
                    for j in range(GP):      # position within group
                        c = slice(g * D + 32 * j, g * D + 32 * j + 32)
                        ck = slice(C + g * D + 32 * j, C + g * D + 32 * j + 32)
                        nc.tensor.matmul(
                            psl[32 * j:32 * j + 32,
                                128 * qq + 32 * t:128 * qq + 32 * t + 32],
                            qk_t[:, ck],
                            qk_t[:, c],
                            start=True, stop=True,
                            tile_position=(0, 32 * j),
                        )

            def emit_exp(hh):
                psl = half_state[hh]
                exp_sb = exp_pool.tile([128, 512], fp16, tag="exp_sb",
                                       name=f"exp_{hh}")
                nc.scalar.activation(
                    exp_sb, psl, mybir.ActivationFunctionType.Exp,
                    scale=_SCALE,
                )
                half_state[hh] = exp_sb

            def emit_wv_quad(hh, qq):
                n, half = divmod(hh, 2)
                _, vp_t, out_t = chunk_tiles[n]
                exp_sb = half_state[hh]
                psum_o = pso_pool.tile([128, 1024], fp32, tag="pso",
                                       name=f"pso_{hh}_{qq}")
                for p2 in range(2):          # pair of groups (one psum bank)
                    for u in range(2):
                        g = half * 16 + qq * 4 + p2 * 2 + u
                        t = p2 * 2 + u
                        for j in range(GP):
                            r = slice(32 * j, 32 * j + 32)
                            nc.tensor.matmul(
                                psum_o[r, 512 * p2 + 129 * u:
                                       512 * p2 + 129 * u + 129],
                                exp_sb[r, 128 * qq + 32 * t:
                                       128 * qq + 32 * t + 32],
                                vp_t[r, g * VCOL:(g + 1) * VCOL],
                                start=True, stop=True,
                                tile_position=(32 * j, 32 * j),
                            )
                recip = small_pool.tile([128, 4], fp32, tag="recip",
                                        name=f"recip_{hh}_{qq}")
                pv = psum_o.rearrange("p (a r) -> p a r", a=2)[:, :, :258]
                pv = pv.rearrange("p a (u c) -> p a u c", u=2)
                rv = recip.rearrange("p (a u) -> p a u", a=2)
                nc.vector.reciprocal(rv, pv[:, :, :, D])

                g0 = half * 16 + qq * 4
                if qq == 1:
                    # Split quad 1: ScalarE copies pair 0 (2 groups), VectorE
                    # multiplies pair 1.  Quad 1, not 0 or 3: with pso bufs=3,
                    # quad 3 reuses quad 0's psum buffer, so quad 0 must be
                    # freed by the fast VectorE path; the slower scalar chain
                    # on quad 1 has until next slot's quad 0.
                    for i in range(2):
                        g = g0 + i
                        src = psum_o[:, 129 * i:129 * i + D]
                        nc.scalar.activation(
                            out_t[:, g * D:(g + 1) * D], src,
                            mybir.ActivationFunctionType.Copy,
                            scale=recip[:, i:i + 1],
                        )
                    dst1 = out_t[:, (g0 + 2) * D:(g0 + 4) * D].rearrange(
                        "p (u c) -> p u c", u=2)
                    mul1 = rv[:, 1, :].unsqueeze(2).broadcast_to((128, 2, D))
                    nc.vector.tensor_tensor(
                        dst1, pv[:, 1, :, 0:D], mul1,
                        op=mybir.AluOpType.mult,
                    )
                else:
                    # VectorE path: one broadcast multiply for 4 groups
                    dst = out_t[:, g0 * D:(g0 + 4) * D].rearrange(
                        "p (a u c) -> p a u c", a=2, u=2)
                    mul = rv.unsqueeze(3).broadcast_to((128, 2, 2, D))
                    nc.vector.tensor_tensor(
                        dst, pv[:, :, :, 0:D], mul,
                        op=mybir.AluOpType.mult,
                    )

            def emit_drain(hh):
                n, half = divmod(hh, 2)
                _, _, out_t = chunk_tiles[n]
                half_state.pop(hh)
                half_c = C // 2
                nc.gpsimd.dma_start(
                    out=out[n, :, half * half_c:(half + 1) * half_c],
                    in_=out_t[:, half * half_c:(half + 1) * half_c])

            # Software pipeline with quad interleave: the PE alternates
            # QK-quad(hh+1) and WV-quad(hh).  The QK matmuls between two WV
            # quads cover the recip->evacuate->psum-free latency, and the
            # weight-load pipe prefetches ahead during the other stream's
            # matmuls, keeping the 4-way column-tiled fast mode engaged.
            # Output drains are deferred DRAIN_DELAY halves so the input
            # prefetch phase gets the full DMA bandwidth; drains then fill
            # the DMA-idle compute tail.
            DRAIN_DELAY = 1
            for hh in range(NHALF + DRAIN_DELAY + 1):
                n, half = divmod(hh, 2)
                if half == 0 and hh < NHALF:
                    emit_load(n)
                for qq in range(4):
                    if hh < NHALF:
                        emit_qk_quad(hh, qq)
                    if 1 <= hh <= NHALF:
                        emit_wv_quad(hh - 1, qq)
                if hh < NHALF:
                    emit_exp(hh)
                if 0 <= hh - DRAIN_DELAY < NHALF:
                    emit_drain(hh - DRAIN_DELAY)

    nc.compile()
    return nc


def _host_pack(q, k, v):
    """Build per-core device input arrays from full fp32 inputs."""
    import ml_dtypes
    f8 = ml_dtypes.float8_e3m4

    qf = np.ascontiguousarray(q, dtype=np.float32).reshape(POS, H, D)
    kf = np.ascontiguousarray(k, dtype=np.float32).reshape(POS, H, D)
    vf = np.ascontiguousarray(v, dtype=np.float32).reshape(POS, H, D)

    nchunk_tot = POS // CHUNK_POS
    # q,k: [chunk, group, i, h, d] -> [chunk, d, (group, i, h)]
    def to_qt(x):
        x = x.reshape(nchunk_tot, NG, GP, H, D)
        x = x.transpose(0, 4, 1, 2, 3)
        return np.ascontiguousarray(x.reshape(nchunk_tot, D, NG * GP * H)).astype(f8)

    qk_all = np.concatenate([to_qt(qf), to_qt(kf)], axis=2)

    # v: [chunk, group, i, gh, d] -> [chunk, (i,gh), (group, d|1)]
    vv = vf.reshape(nchunk_tot, NG, GP, H, D).transpose(0, 2, 3, 1, 4)
    vp_all = np.ones((nchunk_tot, GP, H, NG, VCOL), dtype=np.float32)
    vp_all[..., :D] = vv
    vp_all = np.ascontiguousarray(
        vp_all.reshape(nchunk_tot, GP * H, NG * VCOL)
    ).astype(np.float16)

    in_maps = []
    for c in range(NCORES):
        sl = slice(c * NCHUNK, (c + 1) * NCHUNK)
        in_maps.append({
            "qk": np.ascontiguousarray(qk_all[sl]),
            "vp": np.ascontiguousarray(vp_all[sl]),
        })
    return in_maps


def _host_unpack(outs):
    """Per-core [NCHUNK, 128, C] fp16 -> full [B, S, H*D] fp32."""
    full = np.concatenate(outs, axis=0).astype(np.float32)
    nchunk_tot = POS // CHUNK_POS
    full = full.reshape(nchunk_tot, GP, H, NG, D)   # [chunk, i, h, g, d]
    full = full.transpose(0, 3, 1, 2, 4)            # [chunk, g, i, h, d]
    return np.ascontiguousarray(full.reshape(B, S, H * D))


def kernel(q, k, v, _trace=False):
    global _program
    from concourse.bass_utils import run_bass_kernel_spmd

    if _program is None:
        _program = _build_program()

    in_maps = _host_pack(q, k, v)
    res = run_bass_kernel_spmd(_program, in_maps, list(range(NCORES)), trace=_trace)
    outs = [res.results[c]["out"] for c in range(NCORES)]
    result = _host_unpack(outs)
    if _trace:
        return result, res
    return result


# revision 19
# speedup vs baseline: 1.1176x; 1.0256x over previous
"""Trainium2 Bass kernel for per-position head-attention (nn_DariushFlashAttention2).

Math (per batch b, sequence position s):
    Q = q[b,s].reshape(H=32, D=128); K, V likewise
    logits = Q @ K.T / sqrt(D)          # [32, 32] attention over HEADS
    W = softmax(logits, axis=-1)
    out[b,s] = (W @ V).reshape(H*D)

Every one of the B*S = 8192 positions is independent, so we shard positions
across the 8 NeuronCores (1024 positions each) and run one SPMD program.

Device strategy (per core):
  - Positions are packed 4-per-"group" onto the 128 SBUF partitions
    (partition = 4*32 = pos_in_group x head).
  - Host pre-transposes q,k into [d, (pos,h)] layout and pre-casts to
    fp8-e3m4 (4 mantissa bits keep the overall rel-err ~1.7e-2, under the
    2e-2 gate), packed side by side in ONE dram tensor so each chunk loads
    with a single 8KB-row DMA.  v stays fp16 with a ones-column per group
    (emits the softmax denominator from the same WV matmul).
  - QK: per position one col-tiled fp8 matmul (tile_position=(0,32j),
    K=128(d), M=32(k-heads), N=32(q-heads)) -> psum block-row for that
    position only; 4 quads of groups share one [128,512] psum bank.
  - One exp() per 16 groups on ScalarE over the whole [128,512] psum tile.
  - WV: per position a (32j,32j) sub-array matmul; V is stored
    [(pos,g), d|1] so the same matmul emits the denominator in its last
    column.  4 groups of WV output live in one [128,1024] 2-bank psum tile.
  - One batched reciprocal per 4 groups; normalize-and-evacuate PSUM with
    one broadcast tensor_tensor per 4 groups on VectorE (3 of 4 tiles) or
    per-group scaled copies on ScalarE (1 of 4 tiles) to balance engines.
  - Output halves drain via the GpSimd HWDGE ring so out-DMAs never
    contend with input prefetch or ScalarE dispatch.
"""

import numpy as np

B, S, H, D = 2, 4096, 32, 128
NCORES = 8
POS = B * S                  # 8192 positions total
PPC = POS // NCORES          # 1024 positions per core
GP = 4                       # positions per group (4*32 heads = 128 partitions)
NG = 32                      # groups per chunk
CHUNK_POS = GP * NG          # 128 positions per chunk
NCHUNK = PPC // CHUNK_POS    # 8 chunks per core
VCOL = D + 1                 # v columns per group incl. ones column
C = NG * D                   # 4096 q (or k) columns per chunk

_SCALE = float(1.0 / np.sqrt(D))

_program = None  # cached compiled Bass program


def _build_program():
    import concourse.bacc as bacc
    import concourse.mybir as mybir
    from concourse.tile import TileContext

    fp32 = mybir.dt.float32
    fp16 = mybir.dt.float16
    fp8 = mybir.dt.float8e3

    nc = bacc.Bacc()
    qk = nc.dram_tensor("qk", [NCHUNK, 128, 2 * C], fp8, kind="ExternalInput")
    vp = nc.dram_tensor("vp", [NCHUNK, 128, NG * VCOL], fp16, kind="ExternalInput")
    out = nc.dram_tensor("out", [NCHUNK, 128, C], fp16, kind="ExternalOutput")

    NHALF = NCHUNK * 2
    with TileContext(nc) as tc:
        with (
            tc.tile_pool(name="qk_in", bufs=3) as qk_pool,
            tc.tile_pool(name="v_in", bufs=3) as v_pool,
            tc.tile_pool(name="o_out", bufs=3) as o_pool,
            tc.tile_pool(name="exp", bufs=3) as exp_pool,
            tc.tile_pool(name="small", bufs=8) as small_pool,
            tc.tile_pool(name="psl", bufs=2, space="PSUM") as psl_pool,
            tc.tile_pool(name="pso", bufs=3, space="PSUM") as pso_pool,
        ):
            chunk_tiles = {}   # n -> (qk_t, vp_t, out_t)
            half_state = {}    # hh -> exp_sb

            def emit_load(n):
                qk_t = qk_pool.tile([128, 2 * C], fp8, tag="qk")
                vp_t = v_pool.tile([128, NG * VCOL], fp16, tag="vp")
                nc.sync.dma_start(out=qk_t, in_=qk[n, :, :])
                nc.sync.dma_start(out=vp_t, in_=vp[n, :, :])
                out_t = o_pool.tile([128, C], fp16, tag="out")
                chunk_tiles[n] = (qk_t, vp_t, out_t)

            def emit_qk_quad(hh, qq):
                n, half = divmod(hh, 2)
                qk_t, _, _ = chunk_tiles[n]
                if qq == 0:
                    half_state[hh] = psl_pool.tile(
                        [128, 512], fp32, tag="psl", name=f"psl_{hh}")
                psl = half_state[hh]
                for t in range(4):           # group within quad
                    g = half * 16 + qq * 4 + t
                    for j in range(GP):      # position within group
                        c = slice(g * D + 32 * j, g * D + 32 * j + 32)
                        ck = slice(C + g * D + 32 * j, C + g * D + 32 * j + 32)
                        nc.tensor.matmul(
                            psl[32 * j:32 * j + 32,
                                128 * qq + 32 * t:128 * qq + 32 * t + 32],
                            qk_t[:, ck],
                            qk_t[:, c],
                            start=True, stop=True,
                            tile_position=(0, 32 * j),
                        )

            def emit_exp(hh):
                psl = half_state[hh]
                exp_sb = exp_pool.tile([128, 512], fp16, tag="exp_sb",
                                       name=f"exp_{hh}")
                nc.scalar.activation(
                    exp_sb, psl, mybir.ActivationFunctionType.Exp,
                    scale=_SCALE,
                )
                half_state[hh] = exp_sb

            def emit_wv_quad(hh, qq):
                n, half = divmod(hh, 2)
                _, vp_t, out_t = chunk_tiles[n]
                exp_sb = half_state[hh]
                psum_o = pso_pool.tile([128, 1024], fp32, tag="pso",
                                       name=f"pso_{hh}_{qq}")
                for p2 in range(2):          # pair of groups (one psum bank)
                    for u in range(2):
                        g = half * 16 + qq * 4 + p2 * 2 + u
                        t = p2 * 2 + u
                        for j in range(GP):
                            r = slice(32 * j, 32 * j + 32)
                            nc.tensor.matmul(
                                psum_o[r, 512 * p2 + 129 * u:
                                       512 * p2 + 129 * u + 129],
                                exp_sb[r, 128 * qq + 32 * t:
                                       128 * qq + 32 * t + 32],
                                vp_t[r, g * VCOL:(g + 1) * VCOL],
                                start=True, stop=True,
                                tile_position=(32 * j, 32 * j),
                            )
                recip = small_pool.tile([128, 4], fp32, tag="recip",
                                        name=f"recip_{hh}_{qq}")
                pv = psum_o.rearrange("p (a r) -> p a r", a=2)[:, :, :258]
                pv = pv.rearrange("p a (u c) -> p a u c", u=2)
                rv = recip.rearrange("p (a u) -> p a u", a=2)
                nc.vector.reciprocal(rv, pv[:, :, :, D])

                g0 = half * 16 + qq * 4
                if qq == 1:
                    # Split quad 1: ScalarE copies pair 0 (2 groups), VectorE
                    # multiplies pair 1.  Quad 1, not 0 or 3: with pso bufs=3,
                    # quad 3 reuses quad 0's psum buffer, so quad 0 must be
                    # freed by the fast VectorE path; the slower scalar chain
                    # on quad 1 has until next slot's quad 0.
                    for i in range(2):
                        g = g0 + i
                        src = psum_o[:, 129 * i:129 * i + D]
                        nc.scalar.activation(
                            out_t[:, g * D:(g + 1) * D], src,
                            mybir.ActivationFunctionType.Copy,
                            scale=recip[:, i:i + 1],
                        )
                    dst1 = out_t[:, (g0 + 2) * D:(g0 + 4) * D].rearrange(
                        "p (u c) -> p u c", u=2)
                    mul1 = rv[:, 1, :].unsqueeze(2).broadcast_to((128, 2, D))
                    nc.vector.tensor_tensor(
                        dst1, pv[:, 1, :, 0:D], mul1,
                        op=mybir.AluOpType.mult,
                    )
                else:
                    # VectorE path: one broadcast multiply for 4 groups
                    dst = out_t[:, g0 * D:(g0 + 4) * D].rearrange(
                        "p (a u c) -> p a u c", a=2, u=2)
                    mul = rv.unsqueeze(3).broadcast_to((128, 2, 2, D))
                    nc.vector.tensor_tensor(
                        dst, pv[:, :, :, 0:D], mul,
                        op=mybir.AluOpType.mult,
                    )

            def emit_drain(hh):
                n, half = divmod(hh, 2)
                _, _, out_t = chunk_tiles[n]
                half_state.pop(hh)
                half_c = C // 2
                nc.sync.dma_start(
                    out=out[n, :, half * half_c:(half + 1) * half_c],
                    in_=out_t[:, half * half_c:(half + 1) * half_c])

            # Software pipeline with quad interleave: the PE alternates
            # QK-quad(hh+1) and WV-quad(hh).  The QK matmuls between two WV
            # quads cover the recip->evacuate->psum-free latency, and the
            # weight-load pipe prefetches ahead during the other stream's
            # matmuls, keeping the 4-way column-tiled fast mode engaged.
            # Output drains are deferred DRAIN_DELAY halves so the input
            # prefetch phase gets the full DMA bandwidth; drains then fill
            # the DMA-idle compute tail.
            DRAIN_DELAY = 1
            for hh in range(NHALF + DRAIN_DELAY + 1):
                n, half = divmod(hh, 2)
                if half == 0 and hh < NHALF:
                    emit_load(n)
                for qq in range(4):
                    if hh < NHALF:
                        emit_qk_quad(hh, qq)
                    if 1 <= hh <= NHALF:
                        emit_wv_quad(hh - 1, qq)
                if hh < NHALF:
                    emit_exp(hh)
                if 0 <= hh - DRAIN_DELAY < NHALF:
                    emit_drain(hh - DRAIN_DELAY)

    nc.compile()
    return nc


def _host_pack(q, k, v):
    """Build per-core device input arrays from full fp32 inputs."""
    import ml_dtypes
    f8 = ml_dtypes.float8_e3m4

    qf = np.ascontiguousarray(q, dtype=np.float32).reshape(POS, H, D)
    kf = np.ascontiguousarray(k, dtype=np.float32).reshape(POS, H, D)
    vf = np.ascontiguousarray(v, dtype=np.float32).reshape(POS, H, D)

    nchunk_tot = POS // CHUNK_POS
    # q,k: [chunk, group, i, h, d] -> [chunk, d, (group, i, h)]
    def to_qt(x):
        x = x.reshape(nchunk_tot, NG, GP, H, D)
        x = x.transpose(0, 4, 1, 2, 3)
        return np.ascontiguousarray(x.reshape(nchunk_tot, D, NG * GP * H)).astype(f8)

    qk_all = np.concatenate([to_qt(qf), to_qt(kf)], axis=2)

    # v: [chunk, group, i, gh, d] -> [chunk, (i,gh), (group, d|1)]
    vv = vf.reshape(nchunk_tot, NG, GP, H, D).transpose(0, 2, 3, 1, 4)
    vp_all = np.ones((nchunk_tot, GP, H, NG, VCOL), dtype=np.float32)
    vp_all[..., :D] = vv
    vp_all = np.ascontiguousarray(
        vp_all.reshape(nchunk_tot, GP * H, NG * VCOL)
    ).astype(np.float16)

    in_maps = []
    for c in range(NCORES):
        sl = slice(c * NCHUNK, (c + 1) * NCHUNK)
        in_maps.append({
            "qk": np.ascontiguousarray(qk_all[sl]),
            "vp": np.ascontiguousarray(vp_all[sl]),
        })
    return in_maps


def _host_unpack(outs):
    """Per-core [NCHUNK, 128, C] fp16 -> full [B, S, H*D] fp32."""
    full = np.concatenate(outs, axis=0).astype(np.float32)
    nchunk_tot = POS // CHUNK_POS
    full = full.reshape(nchunk_tot, GP, H, NG, D)   # [chunk, i, h, g, d]
    full = full.transpose(0, 3, 1, 2, 4)            # [chunk, g, i, h, d]
    return np.ascontiguousarray(full.reshape(B, S, H * D))


def kernel(q, k, v, _trace=False):
    global _program
    from concourse.bass_utils import run_bass_kernel_spmd

    if _program is None:
        _program = _build_program()

    in_maps = _host_pack(q, k, v)
    res = run_bass_kernel_spmd(_program, in_maps, list(range(NCORES)), trace=_trace)
    outs = [res.results[c]["out"] for c in range(NCORES)]
    result = _host_unpack(outs)
    if _trace:
        return result, res
    return result
